# revision 8
# baseline (speedup 1.0000x reference)
"""MoE gate (router) kernel for 8 Trainium2 NeuronCores.

Problem: x (4, 4096, 4096) f32, weight (64, 4096) f32 ->
  topk_idx (16384, 8) i32, topk_weight (16384, 8) f32, aux_loss scalar f32.

Sharding: data-parallel over the 16384 tokens, 2048 per core; the router
weight is replicated.

Precision/speed strategy: native f32 matmul on TRN2 runs at 4 cyc/col
(two half-rate passes). Instead the host splits both operands into fp16
hi/lo pairs and the device runs three full-rate fp16 passes
(hi*hi + hi*lo + lo*hi), accumulating in f32 PSUM. The weight is
pre-scaled by 64 so its fp16 lo-part stays out of the subnormal range
(measured logit error 1.8e-7 max vs f64, better than the native f32
path); the 1/64 is folded into the exp's scale parameter. x is handed
to each core pre-transposed ([4096, 2048] hi/lo) so the contraction (H)
lands on partitions; same total bytes as f32 x.

Per core, per 1024-token block: logits.T accumulates into two PSUM
banks (96 fp16 matmuls each, W^T chunk stationary, x chunk moving);
ScalarE evacuates, PE transposes [64,t]->[t,64]; ScalarE computes
p = exp(logits/64) straight from PSUM with the per-token row-sum Z
accumulated for free; VectorE max8/max_index produce the top-8
values+indices; top-8 weights are v8/sum(v8) (softmax Z cancels). For
the aux loss, per-expert score sums (sum_t p/Z) come from a PE matmul
with rhs = 1/Z; the tiny scalar combine happens on the host during
unsharding.
"""

import numpy as np

H = 4096
E = 64
TOPK = 8
NCORES = 8
B = 4
S = 4096
T = B * S
T_CORE = T // NCORES          # 2048 tokens per core
HC = H // 128                 # 32 contraction chunks
BLK = 1024                    # tokens per block
NB = T_CORE // BLK            # 2 blocks
JPB = BLK // 512              # 512-token j-tiles per block
GPB = BLK // 128              # 8 groups of 128 tokens per block
NG = NB * GPB                 # 16 groups per core
WSCALE = 64.0
ALPHA = 0.001

_PROGRAM = None


def _build_program():
    import concourse.mybir as mybir
    import concourse.tile as tile
    from concourse import bacc

    f32 = mybir.dt.float32
    f16 = mybir.dt.float16
    u32 = mybir.dt.uint32

    nc = bacc.Bacc("TRN2", target_bir_lowering=False, debug=False,
                   num_devices=NCORES)
    xh = nc.declare_dram_parameter("xh", [H, T_CORE], f16, isOutput=False)
    xl = nc.declare_dram_parameter("xl", [H, T_CORE], f16, isOutput=False)
    whp = nc.declare_dram_parameter("whp", [128, HC, E], f16, isOutput=False)
    wlp = nc.declare_dram_parameter("wlp", [128, HC, E], f16, isOutput=False)
    idx_out = nc.declare_dram_parameter("idx_out", [128, NG, TOPK], u32,
                                        isOutput=True)
    w_out = nc.declare_dram_parameter("w_out", [128, NG, TOPK], f32,
                                      isOutput=True)
    esum_out = nc.declare_dram_parameter("esum_out", [E, NB * JPB], f32,
                                         isOutput=True)

    EXP = mybir.ActivationFunctionType.Exp
    AXX = mybir.AxisListType.X

    with tile.TileContext(nc) as tc:
        with (
            tc.tile_pool(name="const", bufs=1) as const,
            tc.tile_pool(name="outs", bufs=1) as outs,
            tc.tile_pool(name="x", bufs=10) as xpool,
            tc.tile_pool(name="lt", bufs=3) as lt_pool,
            tc.tile_pool(name="p", bufs=3) as p_pool,
            tc.tile_pool(name="acc", bufs=2, space="PSUM") as psum_acc,
            tc.tile_pool(name="tr", bufs=2, space="PSUM") as psum_tr,
            tc.tile_pool(name="es", bufs=2, space="PSUM") as psum_es,
        ):
            wh_sb = const.tile([128, HC, E], f16)
            wl_sb = const.tile([128, HC, E], f16)
            nc.sync.dma_start(out=wh_sb, in_=whp[:, :, :])
            nc.sync.dma_start(out=wl_sb, in_=wlp[:, :, :])
            ident = const.tile([128, 128], f32)
            from concourse.masks import make_identity
            make_identity(nc, ident)

            # Warm the ACT exp table while the first DMAs are in flight.
            dummy = const.tile([128, 1], f32)
            nc.vector.memset(dummy, 0.0)
            nc.scalar.activation(dummy, dummy, EXP)

            v8 = outs.tile([128, NG, TOPK], f32)
            i8 = outs.tile([128, NG, TOPK], u32)
            w8 = outs.tile([128, NG, TOPK], f32)
            zsum = outs.tile([128, NG], f32)
            zinv = outs.tile([128, NG], f32)
            rsum = outs.tile([128, NG], f32)
            rinv = outs.tile([128, NG], f32)
            esum = outs.tile([E, NB * JPB], f32)

            for b in range(NB):
                t0 = b * BLK
                acc0 = psum_acc.tile([E, 512], f32, tag="acc0")
                acc1 = psum_acc.tile([E, 512], f32, tag="acc1")
                accs = [acc0, acc1]
                for c in range(HC):
                    xhc = xpool.tile([128, BLK], f16, tag="xh")
                    xlc = xpool.tile([128, BLK], f16, tag="xl")
                    nc.sync.dma_start(
                        out=xhc, in_=xh[c * 128:(c + 1) * 128, t0:t0 + BLK])
                    nc.scalar.dma_start(
                        out=xlc, in_=xl[c * 128:(c + 1) * 128, t0:t0 + BLK])
                    for j in range(JPB):
                        js = slice(j * 512, (j + 1) * 512)
                        nc.tensor.matmul(accs[j], lhsT=wh_sb[:, c, :],
                                         rhs=xhc[:, js],
                                         start=(c == 0), stop=False)
                        nc.tensor.matmul(accs[j], lhsT=wh_sb[:, c, :],
                                         rhs=xlc[:, js],
                                         start=False, stop=False)
                        nc.tensor.matmul(accs[j], lhsT=wl_sb[:, c, :],
                                         rhs=xhc[:, js],
                                         start=False, stop=(c == HC - 1))

                for j in range(JPB):
                    lt = lt_pool.tile([E, 512], f32)
                    nc.scalar.copy(out=lt, in_=accs[j])

                    ptr = psum_tr.tile([128, 4, E], f32)
                    for g in range(4):
                        nc.tensor.transpose(ptr[:, g, :],
                                            lt[:, g * 128:(g + 1) * 128],
                                            ident[:E, :E])

                    pp = p_pool.tile([128, 4, E], f32)
                    for g in range(4):
                        gg = b * GPB + j * 4 + g
                        nc.scalar.activation(pp[:, g, :], ptr[:, g, :], EXP,
                                             scale=1.0 / WSCALE,
                                             accum_out=zsum[:, gg:gg + 1])

                    g0 = b * GPB + j * 4
                    gsl = slice(g0, g0 + 4)
                    nc.vector.reciprocal(zinv[:, gsl], zsum[:, gsl])

                    es = psum_es.tile([E, 1], f32)
                    for g in range(4):
                        gg = g0 + g
                        nc.tensor.matmul(es, lhsT=pp[:, g, :],
                                         rhs=zinv[:, gg:gg + 1],
                                         start=(g == 0), stop=(g == 3))
                    nc.scalar.activation(esum[:, b * JPB + j:b * JPB + j + 1],
                                         es,
                                         mybir.ActivationFunctionType.Copy)

                    for g in range(4):
                        gg = g0 + g
                        nc.vector.max(out=v8[:, gg, :], in_=pp[:, g, :])
                        nc.vector.max_index(out=i8[:, gg, :],
                                            in_max=v8[:, gg, :],
                                            in_values=pp[:, g, :])
                    nc.vector.reduce_sum(out=rsum[:, gsl], in_=v8[:, gsl, :],
                                         axis=AXX)
                    nc.vector.reciprocal(rinv[:, gsl], rsum[:, gsl])
                    for g in range(4):
                        gg = g0 + g
                        nc.vector.tensor_scalar_mul(w8[:, gg, :],
                                                    v8[:, gg, :],
                                                    rinv[:, gg:gg + 1])

            nc.sync.dma_start(out=idx_out[:, :, :], in_=i8)
            nc.sync.dma_start(out=w_out[:, :, :], in_=w8)
            nc.sync.dma_start(out=esum_out[:, :], in_=esum)

    nc.compile()
    return nc


def _get_program():
    global _PROGRAM
    if _PROGRAM is None:
        _PROGRAM = _build_program()
    return _PROGRAM


def kernel(x, weight, **_ignored):
    from concourse.bass_utils import run_bass_kernel_spmd

    nc = _get_program()

    xf = np.ascontiguousarray(x, dtype=np.float32).reshape(T, H)
    wsc = np.asarray(weight, dtype=np.float32) * WSCALE
    wh = wsc.astype(np.float16)
    wl = (wsc - wh.astype(np.float32)).astype(np.float16)
    # packed [p, c, e] = W[e, c*128 + p]
    whp = np.ascontiguousarray(wh.reshape(E, HC, 128).transpose(2, 1, 0))
    wlp = np.ascontiguousarray(wl.reshape(E, HC, 128).transpose(2, 1, 0))

    in_maps = []
    for c in range(NCORES):
        xt = xf[c * T_CORE:(c + 1) * T_CORE].T  # [H, T_CORE] view
        xth = np.ascontiguousarray(xt, dtype=np.float16)
        xtl = np.ascontiguousarray(
            (xt - xth.astype(np.float32)).astype(np.float16))
        in_maps.append({"xh": xth, "xl": xtl, "whp": whp, "wlp": wlp})

    res = run_bass_kernel_spmd(nc, in_maps, list(range(NCORES)))

    idx_parts, w_parts, esums = [], [], []
    for c in range(NCORES):
        r = res.results[c]
        idx_parts.append(
            r["idx_out"].transpose(1, 0, 2).reshape(T_CORE, TOPK)
            .astype(np.int32))
        w_parts.append(
            r["w_out"].transpose(1, 0, 2).reshape(T_CORE, TOPK))
        esums.append(r["esum_out"].sum(axis=1, dtype=np.float64))

    topk_idx = np.ascontiguousarray(np.concatenate(idx_parts, axis=0))
    topk_weight = np.ascontiguousarray(
        np.concatenate(w_parts, axis=0).astype(np.float32))

    # aux loss: per batch row b, ce = histogram(top8 ids) / (S*K/E),
    # mean_scores = (sum_t p/Z) / S ; aux = mean_b sum_e ce*ms * alpha.
    shards_per_batch = S // T_CORE
    aux = 0.0
    for b in range(B):
        ids = topk_idx[b * S:(b + 1) * S].ravel()
        counts = np.bincount(ids, minlength=E).astype(np.float64)
        ce = counts / (S * TOPK / E)
        ms = sum(esums[b * shards_per_batch + j]
                 for j in range(shards_per_batch)) / S
        aux += float((ce * ms).sum())
    aux_loss = np.float32(aux / B * ALPHA)

    return topk_idx, topk_weight, aux_loss


# revision 9
# speedup vs baseline: 1.1784x; 1.1784x over previous
"""MoE gate (router) kernel for 8 Trainium2 NeuronCores.

Problem: x (4, 4096, 4096) f32, weight (64, 4096) f32 ->
  topk_idx (16384, 8) i32, topk_weight (16384, 8) f32, aux_loss scalar f32.

Sharding: data-parallel over the 16384 tokens, 2048 per core; the router
weight is replicated.

Precision/speed strategy: native f32 matmul on TRN2 runs at 4 cyc/col
(two half-rate passes). Instead the host splits both operands into fp16
hi/lo pairs and the device runs three full-rate fp16 passes
(hi*hi + hi*lo + lo*hi), accumulating in f32 PSUM. The weight is
pre-scaled by 64 so its fp16 lo-part stays out of the subnormal range
(measured logit error 1.8e-7 max vs f64, better than the native f32
path); the 1/64 is folded into the exp's scale parameter. x is handed
to each core pre-transposed ([4096, 2048] hi/lo) so the contraction (H)
lands on partitions; same total bytes as f32 x.

Per core, per 1024-token block: logits.T accumulates into two PSUM
banks (96 fp16 matmuls each, W^T chunk stationary, x chunk moving);
ScalarE evacuates, PE transposes [64,t]->[t,64]; ScalarE computes
p = exp(logits/64) straight from PSUM with the per-token row-sum Z
accumulated for free; VectorE max8/max_index produce the top-8
values+indices; top-8 weights are v8/sum(v8) (softmax Z cancels). For
the aux loss, per-expert score sums (sum_t p/Z) come from a PE matmul
with rhs = 1/Z; the tiny scalar combine happens on the host during
unsharding.
"""

import numpy as np

H = 4096
E = 64
TOPK = 8
NCORES = 8
B = 4
S = 4096
T = B * S
T_CORE = T // NCORES          # 2048 tokens per core
HC = H // 128                 # 32 contraction chunks
BLK = 1024                    # tokens per block
NB = T_CORE // BLK            # 2 blocks
JPB = BLK // 512              # 512-token j-tiles per block
GPB = BLK // 128              # 8 groups of 128 tokens per block
NG = NB * GPB                 # 16 groups per core
WSCALE = 64.0
ALPHA = 0.001

_PROGRAM = None


def _build_program():
    import concourse.mybir as mybir
    import concourse.tile as tile
    from concourse import bacc

    f32 = mybir.dt.float32
    f16 = mybir.dt.float16
    u32 = mybir.dt.uint32

    nc = bacc.Bacc("TRN2", target_bir_lowering=False, debug=False,
                   num_devices=NCORES)
    xh = nc.declare_dram_parameter("xh", [H, T_CORE], f16, isOutput=False)
    xl = nc.declare_dram_parameter("xl", [H, T_CORE], f16, isOutput=False)
    whp = nc.declare_dram_parameter("whp", [128, HC, E], f16, isOutput=False)
    wlp = nc.declare_dram_parameter("wlp", [128, HC, E], f16, isOutput=False)
    idx_out = nc.declare_dram_parameter("idx_out", [128, NG, TOPK], u32,
                                        isOutput=True)
    w_out = nc.declare_dram_parameter("w_out", [128, NG, TOPK], f32,
                                      isOutput=True)
    esum_out = nc.declare_dram_parameter("esum_out", [E, NB * JPB], f32,
                                         isOutput=True)

    EXP = mybir.ActivationFunctionType.Exp
    AXX = mybir.AxisListType.X

    with tile.TileContext(nc) as tc:
        with (
            tc.tile_pool(name="const", bufs=1) as const,
            tc.tile_pool(name="outs", bufs=1) as outs,
            tc.tile_pool(name="x", bufs=10) as xpool,
            tc.tile_pool(name="lt", bufs=3) as lt_pool,
            tc.tile_pool(name="p", bufs=3) as p_pool,
            tc.tile_pool(name="acc", bufs=2, space="PSUM") as psum_acc,
            tc.tile_pool(name="tr", bufs=2, space="PSUM") as psum_tr,
            tc.tile_pool(name="es", bufs=2, space="PSUM") as psum_es,
        ):
            wh_sb = const.tile([128, HC, E], f16)
            wl_sb = const.tile([128, HC, E], f16)
            nc.sync.dma_start(out=wh_sb, in_=whp[:, :, :])
            nc.sync.dma_start(out=wl_sb, in_=wlp[:, :, :])
            ident = const.tile([128, 128], f32)
            from concourse.masks import make_identity
            make_identity(nc, ident)

            # Warm the ACT exp table while the first DMAs are in flight.
            dummy = const.tile([128, 1], f32)
            nc.vector.memset(dummy, 0.0)
            nc.scalar.activation(dummy, dummy, EXP)

            v8 = outs.tile([128, NG, TOPK], f32)
            i8 = outs.tile([128, NG, TOPK], u32)
            w8 = outs.tile([128, NG, TOPK], f32)
            zsum = outs.tile([128, NG], f32)
            zinv = outs.tile([128, NG], f32)
            rsum = outs.tile([128, NG], f32)
            rinv = outs.tile([128, NG], f32)
            esum = outs.tile([E, NB * JPB], f32)

            for b in range(NB):
                t0 = b * BLK
                acc0 = psum_acc.tile([E, 512], f32, tag="acc0")
                acc1 = psum_acc.tile([E, 512], f32, tag="acc1")
                accs = [acc0, acc1]
                for c in range(HC):
                    xhc = xpool.tile([128, BLK], f16, tag="xh")
                    xlc = xpool.tile([128, BLK], f16, tag="xl")
                    nc.sync.dma_start(
                        out=xhc, in_=xh[c * 128:(c + 1) * 128, t0:t0 + BLK])
                    nc.sync.dma_start(
                        out=xlc, in_=xl[c * 128:(c + 1) * 128, t0:t0 + BLK])
                    for j in range(JPB):
                        js = slice(j * 512, (j + 1) * 512)
                        nc.tensor.matmul(accs[j], lhsT=wh_sb[:, c, :],
                                         rhs=xhc[:, js],
                                         start=(c == 0), stop=False)
                        nc.tensor.matmul(accs[j], lhsT=wh_sb[:, c, :],
                                         rhs=xlc[:, js],
                                         start=False, stop=False)
                        nc.tensor.matmul(accs[j], lhsT=wl_sb[:, c, :],
                                         rhs=xhc[:, js],
                                         start=False, stop=(c == HC - 1))

                for j in range(JPB):
                    lt = lt_pool.tile([E, 512], f32)
                    nc.scalar.copy(out=lt, in_=accs[j])

                    ptr = psum_tr.tile([128, 4, E], f32)
                    for g in range(4):
                        nc.tensor.transpose(ptr[:, g, :],
                                            lt[:, g * 128:(g + 1) * 128],
                                            ident[:E, :E])

                    pp = p_pool.tile([128, 4, E], f32)
                    for g in range(4):
                        gg = b * GPB + j * 4 + g
                        nc.scalar.activation(pp[:, g, :], ptr[:, g, :], EXP,
                                             scale=1.0 / WSCALE,
                                             accum_out=zsum[:, gg:gg + 1])

                    g0 = b * GPB + j * 4
                    gsl = slice(g0, g0 + 4)
                    nc.vector.reciprocal(zinv[:, gsl], zsum[:, gsl])

                    es = psum_es.tile([E, 1], f32)
                    for g in range(4):
                        gg = g0 + g
                        nc.tensor.matmul(es, lhsT=pp[:, g, :],
                                         rhs=zinv[:, gg:gg + 1],
                                         start=(g == 0), stop=(g == 3))
                    nc.scalar.activation(esum[:, b * JPB + j:b * JPB + j + 1],
                                         es,
                                         mybir.ActivationFunctionType.Copy)

                    for g in range(4):
                        gg = g0 + g
                        nc.vector.max(out=v8[:, gg, :], in_=pp[:, g, :])
                        nc.vector.max_index(out=i8[:, gg, :],
                                            in_max=v8[:, gg, :],
                                            in_values=pp[:, g, :])
                    nc.vector.reduce_sum(out=rsum[:, gsl], in_=v8[:, gsl, :],
                                         axis=AXX)
                    nc.vector.reciprocal(rinv[:, gsl], rsum[:, gsl])
                    for g in range(4):
                        gg = g0 + g
                        nc.vector.tensor_scalar_mul(w8[:, gg, :],
                                                    v8[:, gg, :],
                                                    rinv[:, gg:gg + 1])

            nc.sync.dma_start(out=idx_out[:, :, :], in_=i8)
            nc.sync.dma_start(out=w_out[:, :, :], in_=w8)
            nc.sync.dma_start(out=esum_out[:, :], in_=esum)

    nc.compile()
    return nc


def _get_program():
    global _PROGRAM
    if _PROGRAM is None:
        _PROGRAM = _build_program()
    return _PROGRAM


def kernel(x, weight, **_ignored):
    from concourse.bass_utils import run_bass_kernel_spmd

    nc = _get_program()

    xf = np.ascontiguousarray(x, dtype=np.float32).reshape(T, H)
    wsc = np.asarray(weight, dtype=np.float32) * WSCALE
    wh = wsc.astype(np.float16)
    wl = (wsc - wh.astype(np.float32)).astype(np.float16)
    # packed [p, c, e] = W[e, c*128 + p]
    whp = np.ascontiguousarray(wh.reshape(E, HC, 128).transpose(2, 1, 0))
    wlp = np.ascontiguousarray(wl.reshape(E, HC, 128).transpose(2, 1, 0))

    in_maps = []
    for c in range(NCORES):
        xt = xf[c * T_CORE:(c + 1) * T_CORE].T  # [H, T_CORE] view
        xth = np.ascontiguousarray(xt, dtype=np.float16)
        xtl = np.ascontiguousarray(
            (xt - xth.astype(np.float32)).astype(np.float16))
        in_maps.append({"xh": xth, "xl": xtl, "whp": whp, "wlp": wlp})

    res = run_bass_kernel_spmd(nc, in_maps, list(range(NCORES)))

    idx_parts, w_parts, esums = [], [], []
    for c in range(NCORES):
        r = res.results[c]
        idx_parts.append(
            r["idx_out"].transpose(1, 0, 2).reshape(T_CORE, TOPK)
            .astype(np.int32))
        w_parts.append(
            r["w_out"].transpose(1, 0, 2).reshape(T_CORE, TOPK))
        esums.append(r["esum_out"].sum(axis=1, dtype=np.float64))

    topk_idx = np.ascontiguousarray(np.concatenate(idx_parts, axis=0))
    topk_weight = np.ascontiguousarray(
        np.concatenate(w_parts, axis=0).astype(np.float32))

    # aux loss: per batch row b, ce = histogram(top8 ids) / (S*K/E),
    # mean_scores = (sum_t p/Z) / S ; aux = mean_b sum_e ce*ms * alpha.
    shards_per_batch = S // T_CORE
    aux = 0.0
    for b in range(B):
        ids = topk_idx[b * S:(b + 1) * S].ravel()
        counts = np.bincount(ids, minlength=E).astype(np.float64)
        ce = counts / (S * TOPK / E)
        ms = sum(esums[b * shards_per_batch + j]
                 for j in range(shards_per_batch)) / S
        aux += float((ce * ms).sum())
    aux_loss = np.float32(aux / B * ALPHA)

    return topk_idx, topk_weight, aux_loss
